# revision 19
# baseline (speedup 1.0000x reference)
"""Trainium2 Bass kernel for nn_AttentiveHead (segment_reduce) — v9.

Sharding: core k owns graphs [k*256, (k+1)*256); weights replicated; output
gathered on host. No collectives.

v9 vs v8: score path runs in fp8 (e4m3) end to end — h and W1 are uploaded
fp8 and the W1 matmul uses DoubleRow perf mode (2 k-tiles per pass), tanh
writes fp8, and the w2 score matmul is fp8 DoubleRow too.  This halves the
H-major upload (30MB vs 60MB) and roughly halves PE time on the score MLP.
Layout switches to one-graph-per-chunk ([32, L] score tiles), which turns
the softmax into per-partition ops with [32,1] scalars and single exp/fix
activations.  tanh runs on [128, 1024] PSUM windows to amortize the TRN2
SBUF-op erratum.  Pool (sum+att) matmuls stay fp16 from the node-major
upload; max pool is a fp8 DVE tensor_reduce.  Tile order is s-major.

Numerics (vs fp32 reference, measured on CPU): score path fp8 + pools fp16
gives rel err ~3.5e-4 (tolerance 2e-2).
"""

import math
import numpy as np
from contextlib import ExitStack

R = 3
N = 300000
H = 256
G = 2048
NCORES = 8
GLOC = G // NCORES          # 256 graphs per core
GSUB = 32                   # graphs per sub-block (1 graph per score chunk)
NSUB = GLOC // GSUB         # 8 sub-blocks (= count buckets) per (core, rank)

F32 = np.float32
F16 = np.float16


# ---------------------------------------------------------------- host prep

def _prep(inputs):
    import ml_dtypes
    F8 = ml_dtypes.float8_e4m3fn

    h = np.asarray(inputs["h"], dtype=F32)                # [R, N, H]
    batch = np.asarray(inputs["batch"]).astype(np.int64)  # [R, N] sorted

    cnt = np.zeros((R, G), np.int64)
    for r in range(R):
        u, c = np.unique(batch[r], return_counts=True)
        cnt[r, u] = c
    starts = np.zeros((R, G + 1), np.int64)
    starts[:, 1:] = np.cumsum(cnt, 1)
    assert cnt.min() > 0, "empty graph: padding softmax would divide by zero"

    # per-(core, rank) permutation: sort local graphs by that rank's count;
    # rank alignment is restored on device by permutation matmuls.
    perms = [[np.argsort(cnt[r, k * GLOC:(k + 1) * GLOC], kind="stable")
              for r in range(R)] for k in range(NCORES)]

    # bucket pad schedule (shared by all cores — one NEFF). L mult of 4 so
    # SUBN = 32*L is a multiple of 128 (whole node-blocks per sub-block).
    Ls = np.zeros(NSUB, np.int64)
    for k in range(NCORES):
        for r in range(R):
            sk = np.sort(cnt[r, k * GLOC:(k + 1) * GLOC])
            for j in range(NSUB):
                Ls[j] = max(Ls[j], sk[(j + 1) * GSUB - 1])
    Ls = np.maximum(((Ls + 3) // 4) * 4, 8)
    assert Ls.max() <= 256, f"graph too large: L={Ls.max()}"
    SUBNs = (GSUB * Ls).astype(np.int64)
    offs = np.zeros(NSUB + 1, np.int64)
    offs[1:] = np.cumsum(SUBNs)
    NLP = int(offs[-1])
    assert NLP % 128 == 0
    nbs = [int(x) // 128 for x in SUBNs]     # node-blocks per sub-block
    boffs = np.zeros(NSUB + 1, np.int64)
    boffs[1:] = np.cumsum(nbs)
    NBT = int(boffs[-1])                     # total node-blocks (= NLP/128)

    W1 = np.asarray(inputs["W1"], F32)
    b1 = np.asarray(inputs["b1"], F32)
    w2 = np.asarray(inputs["w2"], F32)
    Wp = np.asarray(inputs["Wp"], F32)
    bp = np.asarray(inputs["bp"], F32)
    ln_g = np.asarray(inputs["ln_g"], F32)
    ln_b = np.asarray(inputs["ln_b"], F32)
    Wf1 = np.asarray(inputs["Wf1"], F32)
    bf1 = np.asarray(inputs["bf1"], F32)
    Wf2 = np.asarray(inputs["Wf2"], F32)
    bf2 = np.asarray(inputs["bf2"], F32)

    sigma = [float(np.dot(w2[r], np.tanh(b1[r]))) for r in range(R)]

    # graph one-hot mask per node-block: mask[p, b, j] = 1 iff node
    # b_local*128+p of its sub-block belongs to local graph j (0..31).
    maskc = np.zeros((128, NBT, 32), F16)
    for s in range(NSUB):
        L = int(Ls[s])
        for bl in range(nbs[s]):
            bg = int(boffs[s]) + bl
            n = bl * 128 + np.arange(128)
            maskc[np.arange(128), bg, n // L] = 1.0

    # per-core packed node data
    hp8s, hTds, lmls, rcalls, pmats = [], [], [], [], []
    for k in range(NCORES):
        hp = np.zeros((R, NLP, H), F32)
        lml = np.full((GSUB, R * NSUB), -1e30, F32)
        rc = np.zeros((128, R * 2), F32)
        pmat = np.zeros((128, R * 2 * 2 * 128), F16)
        for r in range(R):
            pm = perms[k][r]
            for p in range(GLOC):
                gl = int(pm[p])
                pmat[p % 128, ((r * 2 + p // 128) * 2 + gl // 128) * 128
                     + gl % 128] = 1.0
                g = k * GLOC + gl
                j = p // GSUB
                q = p % GSUB
                col0 = int(offs[j]) + q * int(Ls[j])
                c = int(cnt[r, g])
                s0 = int(starts[r, g])
                hp[r, col0:col0 + c] = h[r, s0:s0 + c]
                pad = int(Ls[j]) - c
                if pad > 0:
                    lml[q, r * NSUB + j] = math.log(pad) + sigma[r]
                rc[p % 128, r * 2 + p // 128] = 1.0 / max(c, 1)
        # H-major fp8 in DoubleRow k-tile layout: [128p, 2i, n] with
        # H = i*128 + p
        t8 = hp.reshape(R, NLP, 2, 128).transpose(0, 3, 2, 1)
        hp8s.append(np.ascontiguousarray(t8).astype(F8))
        hTds.append(np.ascontiguousarray(
            hp.reshape(R, NBT, 128, H).transpose(0, 2, 1, 3).reshape(
                R, 128, NBT * H)).astype(F16))            # node-major blocks
        lmls.append(lml)
        rcalls.append(rc)
        pmats.append(pmat)

    # weights in device layouts (shared across cores)
    # W1 fp8 DoubleRow stationary: [128p, 2i, (r*2+o)*128 + m]
    w1all8 = np.zeros((128, 2, R * 2 * 128), F8)
    b1all = np.zeros((128, R * 2), F32)
    for r in range(R):
        for o in range(2):
            for i in range(2):
                w1all8[:, i, (r * 2 + o) * 128:(r * 2 + o + 1) * 128] = \
                    W1[r, i * 128:(i + 1) * 128,
                       o * 128:(o + 1) * 128].astype(F8)
            b1all[:, r * 2 + o] = b1[r, o * 128:(o + 1) * 128]

    # w2 fp8 DoubleRow stationary, zero-column selection: chunk c of rank r
    # uses cols [(r*32+c)*32, ...+32) with w2 in column c, zero elsewhere.
    w2sel8 = np.zeros((128, 2, R * GSUB * GSUB), F8)
    for r in range(R):
        for i in range(2):
            w2c = w2[r, i * 128:(i + 1) * 128].astype(F8)
            for c in range(GSUB):
                w2sel8[:, i, (r * GSUB + c) * GSUB + c] = w2c

    # rank-proj: si 0..5 -> prA (sum, max, att), si 6..7 -> prB (mean)
    rows = [(0, 128), (128, 256), (512, 640), (640, 768),
            (768, 896), (896, 1024), (256, 384), (384, 512)]
    wpall = np.zeros((128, R * 8 * 256), F16)
    for r in range(R):
        for si, (a, b) in enumerate(rows):
            wpall[:, (r * 8 + si) * 256:(r * 8 + si + 1) * 256] = \
                Wp[r, a:b, :].astype(F16)
    bpbc = np.zeros((128, R * 256), F16)
    for r in range(R):
        bpbc[:, r * 256:(r + 1) * 256] = bp[r][None, :].astype(F16)

    lngbc = np.broadcast_to(ln_g.astype(F16), (128, R * 256)).copy()
    lnbbc = np.broadcast_to(ln_b.astype(F16), (128, R * 256)).copy()
    wf1 = np.zeros((128, 6 * 256), F16)
    for kb in range(6):
        wf1[:, kb * 256:(kb + 1) * 256] = Wf1[kb * 128:(kb + 1) * 128, :]
    bf1bc = np.broadcast_to(bf1.astype(F16), (128, 256)).copy()
    wf2 = np.zeros((128, 2), F16)
    for kb in range(2):
        wf2[:, kb] = Wf2[kb * 128:(kb + 1) * 128, 0]
    ident = np.eye(128, dtype=F32)
    ident16 = np.eye(128, dtype=F16)

    shared = dict(w1all8=w1all8, w2sel8=w2sel8, b1all=b1all, wpall=wpall,
                  bpbc=bpbc, lngbc=lngbc, lnbbc=lnbbc, wf1=wf1,
                  bf1bc=bf1bc, wf2=wf2, ident=ident, ident16=ident16,
                  maskc=maskc.reshape(128, NBT * 32))
    percore = [dict(hp8=hp8s[k], hTd=hTds[k], lml=lmls[k], rcall=rcalls[k],
                    pmat=pmats[k])
               for k in range(NCORES)]
    meta = dict(Ls=[int(x) for x in Ls],
                SUBNs=[int(x) for x in SUBNs], offs=[int(x) for x in offs],
                nbs=nbs, boffs=[int(x) for x in boffs], NBT=NBT,
                NLP=NLP, sigma=sigma, bf2=float(bf2[0]), perms=perms)
    return shared, percore, meta


# ---------------------------------------------------------------- device IR

def _build(ctx, tc, ins, out_ap, meta):
    import concourse.mybir as mybir

    nc = tc.nc
    dt = mybir.dt
    Act = mybir.ActivationFunctionType
    Alu = mybir.AluOpType
    AX = mybir.AxisListType
    DR = mybir.MatmulPerfMode.DoubleRow

    Ls, SUBNs, offs, nbs, boffs = (
        meta[k] for k in ("Ls", "SUBNs", "offs", "nbs", "boffs"))
    SUBN_MAX = max(SUBNs)
    L_MAX = max(Ls)
    NB_MAX = max(nbs)
    NBH_MAX = (NB_MAX + 1) // 2

    cpool = ctx.enter_context(tc.tile_pool(name="const", bufs=1))
    hpool = ctx.enter_context(tc.tile_pool(name="hp8", bufs=1))
    tpool = ctx.enter_context(tc.tile_pool(name="hT", bufs=3))
    mpool = ctx.enter_context(tc.tile_pool(name="mask", bufs=1))
    thpool = ctx.enter_context(tc.tile_pool(name="th8", bufs=1))
    spool = ctx.enter_context(tc.tile_pool(name="small", bufs=2))
    wapool = ctx.enter_context(tc.tile_pool(name="wall", bufs=2))
    rpool = ctx.enter_context(tc.tile_pool(name="rank", bufs=1))
    fpool = ctx.enter_context(tc.tile_pool(name="final", bufs=1))
    psx = ctx.enter_context(tc.tile_pool(name="psx", bufs=1, space="PSUM"))
    pss = ctx.enter_context(tc.tile_pool(name="pss", bufs=1, space="PSUM"))
    psp = ctx.enter_context(tc.tile_pool(name="psp", bufs=1, space="PSUM"))
    psr = ctx.enter_context(tc.tile_pool(name="psr", bufs=1, space="PSUM"))

    def const_tile(name):
        ap = ins[name]
        t = cpool.tile(list(ap.shape), ap.dtype, tag=name, name=name)
        nc.sync.dma_start(t[:], ap)
        return t

    w1all8 = const_tile("w1all8")    # [128, 2, R*2*128] fp8
    w2sel8 = const_tile("w2sel8")    # [128, 2, R*32*32] fp8
    b1all = const_tile("b1all")
    wpall = const_tile("wpall")
    bpbc = const_tile("bpbc")
    lngbc = const_tile("lngbc")
    lnbbc = const_tile("lnbbc")
    wf1 = const_tile("wf1")
    bf1bc = const_tile("bf1bc")
    wf2 = const_tile("wf2")
    ident = const_tile("ident")
    ident16 = const_tile("ident16")
    lml = const_tile("lml")          # [32, R*NSUB] f32
    rcall = const_tile("rcall")
    pmat = const_tile("pmat")        # [128, R*2*2*128] fp16
    maskd = ins["maskc"]             # [128, NBT*32] fp16 dram (streamed)

    hp8d = ins["hp8"]   # [R, 128, 2, NLP] fp8 dram
    hTd = ins["hTd"]    # [R, 128, NBT*256] fp16 dram
    NBP_MAX = (NB_MAX + 15) // 16 * 16
    scr = nc.dram_tensor(f"scratch{nc.next_id()}", (2, NBP_MAX * 128),
                         dt.float16, kind="Internal").ap()

    state = [fpool.tile([128, 3 * 256], dt.float16, tag=f"state{gh}",
                        name=f"state{gh}")
             for gh in range(2)]

    T = R * NSUB                     # tiles, s-major: t = s*R + r
    hp8_t, hT_t, th8_t, psS_t, wall_t, mask_t = {}, {}, {}, {}, {}, {}
    pools_r = {}

    def new_rank_pools(r):
        SM = [rpool.tile([128, 256], dt.float16, tag=f"sm{r}_{i}",
                         name=f"sm{r}_{i}") for i in range(2)]
        MX = [rpool.tile([128, 256], dt.float16, tag=f"mx{r}_{i}",
                         name=f"mx{r}_{i}") for i in range(2)]
        AT = [rpool.tile([128, 256], dt.float16, tag=f"at{r}_{i}",
                         name=f"at{r}_{i}") for i in range(2)]
        return SM, MX, AT

    def dma_hp8(t):
        s, r = t // R, t % R
        SUBN, off = SUBNs[s], offs[s]
        hp8 = hpool.tile([128, 2, SUBN_MAX], dt.float8e4, tag=f"hp8{t % 2}",
                         name=f"hp8{t % 2}", bufs=1)
        nc.sync.dma_start(hp8[:, :, :SUBN], hp8d[r, :, :, off:off + SUBN])
        hp8_t[t] = hp8

    def dma_mask(s):
        nb, boff = nbs[s], boffs[s]
        mk = mpool.tile([128, NB_MAX * 32], dt.float16, tag=f"mask{s % 2}",
                        name=f"mask{s % 2}", bufs=1)
        nc.sync.dma_start(mk[:, :nb * 32], maskd[:, boff * 32:(boff + nb) * 32])
        mask_t[s] = mk

    def dma_hT(t):
        s, r = t // R, t % R
        nb, boff = nbs[s], boffs[s]
        hT = tpool.tile([128, NB_MAX * 256], dt.float16, tag="hT", name="hT")
        nc.sync.dma_start(hT[:, :nb * 256],
                          hTd[r, :, boff * 256:(boff + nb) * 256])
        hT_t[t] = hT

    def pe_tile(t):
        """W1 matmuls (fp8 DoubleRow) + tanh into fp8 th tile."""
        s, r = t // R, t % R
        SUBN = SUBNs[s]
        hp8 = hp8_t[t]
        th8 = thpool.tile([128, 2, SUBN_MAX], dt.float8e4, tag=f"th8{t % 2}",
                          name=f"th8{t % 2}", bufs=1)
        for o in range(2):
            base = (r * 2 + o) * 128
            c0 = 0
            while c0 < SUBN:
                c1 = min(SUBN, c0 + 1024)
                px = psx.tile([128, 1024], dt.float32,
                              tag=f"psx{(c0 // 1024) % 2}", bufs=1)
                m0 = min(c0 + 512, c1)
                nc.tensor.matmul(px[:, :m0 - c0],
                                 w1all8[:, :, base:base + 128],
                                 hp8[:, :, c0:m0], start=True, stop=True,
                                 perf_mode=DR)
                if c1 > m0:
                    nc.tensor.matmul(px[:, 512:512 + c1 - m0],
                                     w1all8[:, :, base:base + 128],
                                     hp8[:, :, m0:c1], start=True, stop=True,
                                     perf_mode=DR)
                nc.scalar.activation(th8[:, o, c0:c1], px[:, :c1 - c0],
                                     Act.Tanh,
                                     bias=b1all[:, r * 2 + o:r * 2 + o + 1])
                c0 = c1
        th8_t[t] = th8

    def w2_tile(t):
        """Score matmuls for tile t (issued one stage later)."""
        s, r = t // R, t % R
        L = Ls[s]
        th8 = th8_t.pop(t)
        pssT = pss.tile([32, 2, 256], dt.float32, tag="pss", bufs=1)
        psS = pssT[:, t % 2, :L_MAX]
        for c in range(GSUB):
            sel = (r * GSUB + c) * GSUB
            nc.tensor.matmul(psS[:, :L], w2sel8[:, :, sel:sel + GSUB],
                             th8[:, :, c * L:(c + 1) * L],
                             start=(c == 0), stop=(c == GSUB - 1),
                             perf_mode=DR, skip_group_check=True)
        psS_t[t] = psS

    def softmax_part(t):
        """Softmax for tile t, then eT + w_all construction."""
        s, r = t // R, t % R
        L, SUBN, nb, boff = Ls[s], SUBNs[s], nbs[s], boffs[s]
        psS = psS_t.pop(t)
        negm = spool.tile([32, 1], dt.float32, tag="negm")
        nc.vector.tensor_reduce(negm[:], psS[:, :L], axis=AX.X, op=Alu.max,
                                negate=True)
        e16 = spool.tile([32, L_MAX], dt.float16, tag="e16", bufs=1)
        nc.scalar.activation(e16[:, :L], psS[:, :L], Act.Exp, bias=negm[:])
        den = spool.tile([32, 1], dt.float32, tag="den")
        nc.vector.tensor_reduce(den[:], e16[:, :L], axis=AX.X, op=Alu.add)
        idx = r * NSUB + s
        fix = spool.tile([32, 1], dt.float32, tag="fix")
        nc.scalar.activation(fix[:], negm[:], Act.Exp,
                             bias=lml[:, idx:idx + 1])
        dent = spool.tile([32, 1], dt.float32, tag="dent")
        nc.vector.tensor_tensor(dent[:], den[:], fix[:], op=Alu.subtract)
        rden = spool.tile([32, 1], dt.float32, tag="rden")
        nc.vector.reciprocal(rden[:], dent[:])
        wsb = spool.tile([32, L_MAX], dt.float16, tag="wsb", bufs=1)
        nc.vector.tensor_scalar_mul(wsb[:, :L], e16[:, :L], rden[:])
        # store graph-major rows to DRAM (node order), then fold back to
        # [128 x nb] columns (node b*128+p) via the xbar transpose; these
        # ride the ACT HWDGE queue so they never wait behind bulk uploads
        nc.scalar.dma_start(
            scr[t % 2:t % 2 + 1, :SUBN].rearrange("x (c f) -> (x c) f", f=L),
            wsb[:, :L])
        nbp = (nb + 15) // 16 * 16
        eT = spool.tile([128, NBP_MAX], dt.float16, tag="eT")
        nc.scalar.dma_start_transpose(
            eT[:, :nbp],
            scr[t % 2:t % 2 + 1, :nbp * 128].rearrange("x (b q) -> (x b) q",
                                                       q=128))
        # w_all[:, b*64:(b+1)*64]: [mask | mask*eT[:,b]]
        w_all = wapool.tile([128, NB_MAX * 64], dt.float16, tag="wall")
        wv = w_all[:, :nb * 64].rearrange("p (b j) -> p b j", j=64)
        mv = mask_t[s][:, :nb * 32].rearrange("p (b j) -> p b j", j=32)
        nc.vector.tensor_copy(wv[:, :, 0:32], mv)
        ev = eT[:, :nb].unsqueeze(-1).to_broadcast([128, nb, 32])
        nc.gpsimd.tensor_tensor(wv[:, :, 32:64], mv, ev, op=Alu.mult)
        wall_t[t] = w_all

    def max_tile(t):
        s, r = t // R, t % R
        L, SUBN = Ls[s], SUBNs[s]
        g0 = s * GSUB
        if r not in pools_r:
            pools_r[r] = new_rank_pools(r)
        SM, MX, AT = pools_r[r]
        hp8 = hp8_t.pop(t)
        for i in range(2):
            hv = hp8[:, i, :SUBN].rearrange("p (g l) -> p g l", l=L)
            nc.vector.tensor_reduce(MX[i][:, g0:g0 + GSUB], hv,
                                    axis=AX.X, op=Alu.max)

    def pool_mm(t):
        """Sum+att pooling matmuls for tile t, plus transposes back."""
        s, r = t // R, t % R
        nb = nbs[s]
        g0 = s * GSUB
        w_all = wall_t.pop(t)
        hT = hT_t.pop(t)
        SM, MX, AT = pools_r[r]
        pp = psp.tile([64, 256], dt.float32, tag="pp")
        for b in range(nb):
            nc.tensor.matmul(pp[:], w_all[:, b * 64:(b + 1) * 64],
                             hT[:, b * 256:(b + 1) * 256],
                             start=(b == 0), stop=(b == nb - 1))
        pc = spool.tile([64, 256], dt.float32, tag="pc")
        nc.vector.tensor_copy(pc[:], pp[:])
        ptr = psp.tile([128, 128], dt.float32, tag="ptr", bufs=1)
        for hh in range(2):
            nc.tensor.matmul(ptr[:, hh * 64:(hh + 1) * 64],
                             pc[:, hh * 128:(hh + 1) * 128],
                             ident[:64, :64], is_transpose=True)
            nc.scalar.copy(SM[hh][:, g0:g0 + GSUB],
                           ptr[:, hh * 64:hh * 64 + 32])
            nc.scalar.copy(AT[hh][:, g0:g0 + GSUB],
                           ptr[:, hh * 64 + 32:hh * 64 + 64])

    def rank_tail(r):
        SM, MX, AT = pools_r.pop(r)
        pools6 = [SM[0], SM[1], MX[0], MX[1], AT[0], AT[1]]
        t16 = []
        for gh in range(2):
            prT = psr.tile([128, 2, 256], dt.float32, tag="prAB", bufs=1)
            prA, prB = prT[:, 0, :], prT[:, 1, :]
            for si in range(6):
                nc.tensor.matmul(prA[:], pools6[si][:, gh * 128:(gh + 1) * 128],
                                 wpall[:, (r * 8 + si) * 256:(r * 8 + si + 1) * 256],
                                 start=(si == 0), stop=(si == 5))
            for si in (6, 7):
                nc.tensor.matmul(prB[:],
                                 pools6[si - 6][:, gh * 128:(gh + 1) * 128],
                                 wpall[:, (r * 8 + si) * 256:(r * 8 + si + 1) * 256],
                                 start=(si == 6), stop=(si == 7))
            tmp = fpool.tile([128, 256], dt.float32, tag="prtmp", bufs=1)
            nc.vector.tensor_scalar_mul(tmp[:], prB[:],
                                        rcall[:, r * 2 + gh:r * 2 + gh + 1])
            nc.vector.tensor_tensor(tmp[:], tmp[:], prA[:], op=Alu.add)
            tg = fpool.tile([128, 256], dt.float16, tag=f"t16_{gh}", bufs=1,
                            name=f"t16_{gh}")
            nc.vector.tensor_copy(tg[:], tmp[:])
            t16.append(tg)
        # un-permute graph partitions: state[ghp] = sum_gh pmat[gh->ghp]^T @ t16[gh]
        for ghp in range(2):
            prT = psr.tile([128, 2, 256], dt.float32, tag="prAB", bufs=1)
            pperm = prT[:, 0, :]
            for gh in range(2):
                col = ((r * 2 + gh) * 2 + ghp) * 128
                nc.tensor.matmul(pperm[:], pmat[:, col:col + 128], t16[gh][:],
                                 start=(gh == 0), stop=(gh == 1))
            nc.vector.tensor_tensor(state[ghp][:, r * 256:(r + 1) * 256],
                                    pperm[:], bpbc[:, r * 256:(r + 1) * 256],
                                    op=Alu.add)

    # ------------------------------------------------- software pipeline
    # stage t: dma_hp8(t+1) | w2(t-1) -> softmax(t-1) | pe(t) | max(t) |
    #          pool_mm(t-1) | dma_hT(t+1)
    # w2->softmax run early so the w_all DMA-fold chain for tile t-1
    # completes while pe(t) streams; pool_mm(t-1) then runs stall-free.
    # dma_hT(t+1) is issued after pool_mm(t-1) so the 2-buffer hT rotation
    # never reclaims a buffer before its reader is on the queue.
    dma_mask(0)
    dma_hp8(0)
    dma_hT(0)
    for t in range(T):
        if t + 1 < T:
            dma_hp8(t + 1)
            if (t + 1) % R == 0:
                dma_mask((t + 1) // R)
        if t > 0:
            w2_tile(t - 1)
            softmax_part(t - 1)
        pe_tile(t)
        max_tile(t)
        if t > 1:
            pool_mm(t - 2)
        if t + 1 < T:
            dma_hT(t + 1)
    w2_tile(T - 1)
    softmax_part(T - 1)
    if T > 1:
        pool_mm(T - 2)
    pool_mm(T - 1)
    for r in range(R):
        rank_tail(r)

    # ------------------------------------- final MLP (LN->SiLU->L->SiLU->L)
    D = 3 * 256
    for gh in range(2):
        st = state[gh]
        mu = fpool.tile([128, 1], dt.float32, tag="mu")
        nc.vector.tensor_reduce(mu[:], st[:], axis=AX.X, op=Alu.add)
        nc.vector.tensor_scalar_mul(mu[:], mu[:], 1.0 / D)
        xm = fpool.tile([128, D], dt.float32, tag="xm")
        nc.vector.tensor_scalar(xm[:], st[:], mu[:], None, op0=Alu.subtract)
        y = fpool.tile([128, D], dt.float32, tag="y")
        varsum = fpool.tile([128, 1], dt.float32, tag="vs")
        nc.scalar.activation(y[:], xm[:], Act.Square, accum_out=varsum[:])
        sdv = fpool.tile([128, 1], dt.float32, tag="sdv")
        nc.vector.tensor_scalar(sdv[:], varsum[:], 1.0 / D, 1e-5,
                                op0=Alu.mult, op1=Alu.add)
        nc.scalar.activation(sdv[:], sdv[:], Act.Sqrt)
        rstd = fpool.tile([128, 1], dt.float32, tag="rstd")
        nc.vector.reciprocal(rstd[:], sdv[:])
        nc.vector.tensor_scalar_mul(y[:], xm[:], rstd[:])
        nc.vector.tensor_tensor(y[:], y[:], lngbc[:], op=Alu.mult)
        nc.vector.tensor_tensor(y[:], y[:], lnbbc[:], op=Alu.add)
        nc.scalar.activation(xm[:], y[:], Act.Sigmoid)
        nc.vector.tensor_mul(xm[:], xm[:], y[:])
        x2 = xm

        pf = psx.tile([128, 1024], dt.float32, tag="psx0", bufs=1)
        for kb in range(6):
            pt = psx.tile([128, 1024], dt.float32, tag="psx1", bufs=1)
            nc.tensor.matmul(pt[:, :128], x2[:, kb * 128:(kb + 1) * 128],
                             ident[:], is_transpose=True)
            xT = fpool.tile([128, 128], dt.float16, tag=f"xT{kb}")
            nc.scalar.copy(xT[:], pt[:, :128])
            nc.tensor.matmul(pf[:, :256], xT[:], wf1[:, kb * 256:(kb + 1) * 256],
                             start=(kb == 0), stop=(kb == 5))
        xf = fpool.tile([128, 256], dt.float32, tag="xf")
        nc.vector.tensor_tensor(xf[:], pf[:, :256], bf1bc[:], op=Alu.add)
        xs = fpool.tile([128, 256], dt.float32, tag="xs")
        nc.scalar.activation(xs[:], xf[:], Act.Sigmoid)
        nc.vector.tensor_mul(xf[:], xf[:], xs[:])

        poT = psr.tile([128, 2, 256], dt.float32, tag="prAB", bufs=1)
        po = poT[:, 0, :]
        for kb in range(2):
            pt = psx.tile([128, 1024], dt.float32, tag="psx1", bufs=1)
            nc.tensor.matmul(pt[:, :128], xf[:, kb * 128:(kb + 1) * 128],
                             ident[:], is_transpose=True)
            xT = fpool.tile([128, 128], dt.float16, tag=f"xfT{kb}")
            nc.scalar.copy(xT[:], pt[:, :128])
            nc.tensor.matmul(po[:, :1], xT[:], wf2[:, kb:kb + 1],
                             start=(kb == 0), stop=(kb == 1))
        osb = fpool.tile([128, 1], dt.float32, tag=f"osb{gh}")
        nc.vector.tensor_scalar_add(osb[:], po[:, :1], meta["bf2"])
        nc.sync.dma_start(out_ap[gh], osb[:])


# ---------------------------------------------------------------- driver

def _make_nc(shared, percore, meta, reps=1):
    import concourse.bacc as bacc
    import concourse.mybir as mybir
    from concourse import tile

    nc = bacc.Bacc("TRN2", target_bir_lowering=False, debug=False,
                   enable_asserts=False, num_devices=NCORES)
    ins = {}
    for name, arr in {**shared, **percore[0]}.items():
        ins[name] = nc.dram_tensor(name, arr.shape,
                                   mybir.dt.from_np(arr.dtype),
                                   kind="ExternalInput").ap()
    out_ap = nc.dram_tensor("out", (2, 128, 1), mybir.dt.float32,
                            kind="ExternalOutput").ap()
    with tile.TileContext(nc, trace_sim=False) as t:
        for _ in range(reps):
            with ExitStack() as ctx:
                _build(ctx, t, ins, out_ap, meta)
    nc.compile()
    return nc


LAST_EXEC_NS = None


def _gather_out(results, meta):
    out = np.zeros((G,), F32)
    for k in range(NCORES):
        out[k * GLOC:(k + 1) * GLOC] = results[k]["out"].reshape(256)
    return out


def _run_timed(nc, in_maps, reps):
    """Run via pjrt with inputs device-resident; derive per-rep device time
    from the marginal async per-call time (tunnel overhead ~1ms cancels)."""
    import time
    import jax
    from jax.sharding import Mesh, PartitionSpec, NamedSharding
    from jax.experimental.shard_map import shard_map
    from concourse import bass2jax
    import concourse.mybir as mybir

    bass2jax.install_neuronx_cc_hook()
    n_cores = len(in_maps)
    in_names, out_names, out_avals = [], [], []
    for alloc in nc.m.functions[0].allocations:
        if not isinstance(alloc, mybir.MemoryLocationSet):
            continue
        if not alloc.memorylocations:
            continue
        name = alloc.memorylocations[0].name
        pname = (nc.partition_id_tensor.name
                 if nc.partition_id_tensor else None)
        if alloc.kind == "ExternalInput":
            if name != pname:
                in_names.append(name)
        elif alloc.kind == "ExternalOutput":
            out_names.append(name)
            out_avals.append(jax.core.ShapedArray(
                tuple(alloc.tensor_shape), mybir.dt.np(alloc.dtype)))
    n_params = len(in_names)
    in_names = in_names + out_names
    if nc.partition_id_tensor is not None:
        in_names.append(nc.partition_id_tensor.name)

    def _body(*args):
        operands = list(args)
        if nc.partition_id_tensor is not None:
            operands.append(bass2jax.partition_id_tensor())
        outs = bass2jax._bass_exec_p.bind(
            *operands, out_avals=tuple(out_avals), in_names=tuple(in_names),
            out_names=tuple(out_names), lowering_input_output_aliases=(),
            sim_require_finite=True, sim_require_nnan=True, nc=nc)
        return tuple(outs)

    devices = jax.devices()[:n_cores]
    mesh = Mesh(np.asarray(devices), ("core",))
    nio = n_params + len(out_names)
    sharded = jax.jit(shard_map(_body, mesh=mesh,
                                in_specs=(PartitionSpec("core"),) * nio,
                                out_specs=(PartitionSpec("core"),) * len(out_names),
                                check_rep=False), keep_unused=True)
    sh = NamedSharding(mesh, PartitionSpec("core"))
    concat_in = [jax.device_put(np.concatenate(
        [np.asarray(in_maps[c][nm]) for c in range(n_cores)], axis=0), sh)
        for nm in in_names[:n_params]]
    zeros = [jax.device_put(np.zeros((n_cores * a.shape[0],) + a.shape[1:],
                                     a.dtype), sh) for a in out_avals]
    outs = sharded(*concat_in, *zeros)
    jax.block_until_ready(outs)

    best = None
    if reps > 1:
        def async_total(n):
            jax.block_until_ready(sharded(*concat_in, *zeros))
            t0 = time.perf_counter()
            rs = [sharded(*concat_in, *zeros) for _ in range(n)]
            jax.block_until_ready(rs)
            return time.perf_counter() - t0
        for _ in range(3):
            marg = (async_total(24) - async_total(4)) / 20.0
            if best is None or marg < best:
                best = marg
        best = max(0.0, (best - 1.03e-3)) / reps  # subtract dispatch overhead
    out_np = [np.asarray(o) for o in outs]
    results = []
    for c in range(n_cores):
        m = {}
        for i, nm in enumerate(out_names):
            per = out_avals[i].shape[0]
            m[nm] = out_np[i][c * per:(c + 1) * per]
        results.append(m)
    return results, best


def kernel(**inputs):
    global LAST_EXEC_NS
    import os
    shared, percore, meta = _prep(inputs)
    in_maps = [{**shared, **percore[k]} for k in range(NCORES)]

    reps = int(os.environ.get("KERNEL_TIME_REPS", "0"))
    if reps > 1:
        # timing mode: unroll the body so device time dominates the tunnel
        try:
            nc = _make_nc(shared, percore, meta, reps=reps)
            results, dev_s = _run_timed(nc, in_maps, reps)
            if dev_s is not None:
                LAST_EXEC_NS = int(dev_s * 1e9)
                print(f"measured device time: {dev_s*1e6:.1f} us/rep")
            return _gather_out(results, meta)
        except Exception as e:
            print(f"timing mode failed ({e!r}); falling back to single run")

    nc = _make_nc(shared, percore, meta)
    from concourse import bass_utils
    res = bass_utils.run_bass_kernel_spmd(nc, in_maps,
                                          core_ids=list(range(NCORES)))
    if getattr(res, "exec_time_ns", None):
        LAST_EXEC_NS = int(res.exec_time_ns)
    return _gather_out(res.results, meta)


# revision 20
# speedup vs baseline: 1.1870x; 1.1870x over previous
"""Trainium2 Bass kernel for nn_AttentiveHead (segment_reduce) — v9.

Sharding: core k owns graphs [k*256, (k+1)*256); weights replicated; output
gathered on host. No collectives.

v9 vs v8: score path runs in fp8 (e4m3) end to end — h and W1 are uploaded
fp8 and the W1 matmul uses DoubleRow perf mode (2 k-tiles per pass), tanh
writes fp8, and the w2 score matmul is fp8 DoubleRow too.  This halves the
H-major upload (30MB vs 60MB) and roughly halves PE time on the score MLP.
Layout switches to one-graph-per-chunk ([32, L] score tiles), which turns
the softmax into per-partition ops with [32,1] scalars and single exp/fix
activations.  tanh runs on [128, 1024] PSUM windows to amortize the TRN2
SBUF-op erratum.  Pool (sum+att) matmuls stay fp16 from the node-major
upload; max pool is a fp8 DVE tensor_reduce.  Tile order is s-major.

Numerics (vs fp32 reference, measured on CPU): score path fp8 + pools fp16
gives rel err ~3.5e-4 (tolerance 2e-2).
"""

import math
import numpy as np
from contextlib import ExitStack

R = 3
N = 300000
H = 256
G = 2048
NCORES = 8
GLOC = G // NCORES          # 256 graphs per core
GSUB = 32                   # graphs per sub-block (1 graph per score chunk)
NSUB = GLOC // GSUB         # 8 sub-blocks (= count buckets) per (core, rank)

F32 = np.float32
F16 = np.float16


# ---------------------------------------------------------------- host prep

def _prep(inputs):
    import ml_dtypes
    F8 = ml_dtypes.float8_e4m3fn

    h = np.asarray(inputs["h"], dtype=F32)                # [R, N, H]
    batch = np.asarray(inputs["batch"]).astype(np.int64)  # [R, N] sorted

    cnt = np.zeros((R, G), np.int64)
    for r in range(R):
        u, c = np.unique(batch[r], return_counts=True)
        cnt[r, u] = c
    starts = np.zeros((R, G + 1), np.int64)
    starts[:, 1:] = np.cumsum(cnt, 1)
    assert cnt.min() > 0, "empty graph: padding softmax would divide by zero"

    # per-(core, rank) permutation: sort local graphs by that rank's count;
    # rank alignment is restored on device by permutation matmuls.
    perms = [[np.argsort(cnt[r, k * GLOC:(k + 1) * GLOC], kind="stable")
              for r in range(R)] for k in range(NCORES)]

    # bucket pad schedule (shared by all cores — one NEFF). L mult of 4 so
    # SUBN = 32*L is a multiple of 128 (whole node-blocks per sub-block).
    Ls = np.zeros(NSUB, np.int64)
    for k in range(NCORES):
        for r in range(R):
            sk = np.sort(cnt[r, k * GLOC:(k + 1) * GLOC])
            for j in range(NSUB):
                Ls[j] = max(Ls[j], sk[(j + 1) * GSUB - 1])
    Ls = np.maximum(((Ls + 3) // 4) * 4, 8)
    assert Ls.max() <= 256, f"graph too large: L={Ls.max()}"
    SUBNs = (GSUB * Ls).astype(np.int64)
    offs = np.zeros(NSUB + 1, np.int64)
    offs[1:] = np.cumsum(SUBNs)
    NLP = int(offs[-1])
    assert NLP % 128 == 0
    nbs = [int(x) // 128 for x in SUBNs]     # node-blocks per sub-block
    boffs = np.zeros(NSUB + 1, np.int64)
    boffs[1:] = np.cumsum(nbs)
    NBT = int(boffs[-1])                     # total node-blocks (= NLP/128)

    W1 = np.asarray(inputs["W1"], F32)
    b1 = np.asarray(inputs["b1"], F32)
    w2 = np.asarray(inputs["w2"], F32)
    Wp = np.asarray(inputs["Wp"], F32)
    bp = np.asarray(inputs["bp"], F32)
    ln_g = np.asarray(inputs["ln_g"], F32)
    ln_b = np.asarray(inputs["ln_b"], F32)
    Wf1 = np.asarray(inputs["Wf1"], F32)
    bf1 = np.asarray(inputs["bf1"], F32)
    Wf2 = np.asarray(inputs["Wf2"], F32)
    bf2 = np.asarray(inputs["bf2"], F32)

    sigma = [float(np.dot(w2[r], np.tanh(b1[r]))) for r in range(R)]

    # graph one-hot mask per node-block: mask[p, b, j] = 1 iff node
    # b_local*128+p of its sub-block belongs to local graph j (0..31).
    maskc = np.zeros((128, NBT, 32), F16)
    for s in range(NSUB):
        L = int(Ls[s])
        for bl in range(nbs[s]):
            bg = int(boffs[s]) + bl
            n = bl * 128 + np.arange(128)
            maskc[np.arange(128), bg, n // L] = 1.0

    # per-core packed node data
    hp8s, hTds, lmls, rcalls, pmats = [], [], [], [], []
    for k in range(NCORES):
        hp = np.zeros((R, NLP, H), F32)
        lml = np.full((GSUB, R * NSUB), -1e30, F32)
        rc = np.zeros((128, R * 2), F32)
        pmat = np.zeros((128, R * 2 * 2 * 128), F16)
        for r in range(R):
            pm = perms[k][r]
            for p in range(GLOC):
                gl = int(pm[p])
                pmat[p % 128, ((r * 2 + p // 128) * 2 + gl // 128) * 128
                     + gl % 128] = 1.0
                g = k * GLOC + gl
                j = p // GSUB
                q = p % GSUB
                col0 = int(offs[j]) + q * int(Ls[j])
                c = int(cnt[r, g])
                s0 = int(starts[r, g])
                hp[r, col0:col0 + c] = h[r, s0:s0 + c]
                pad = int(Ls[j]) - c
                if pad > 0:
                    lml[q, r * NSUB + j] = math.log(pad) + sigma[r]
                rc[p % 128, r * 2 + p // 128] = 1.0 / max(c, 1)
        # H-major fp8 in DoubleRow k-tile layout: [128p, 2i, n] with
        # H = i*128 + p
        t8 = hp.reshape(R, NLP, 2, 128).transpose(0, 3, 2, 1)
        hp8s.append(np.ascontiguousarray(t8).astype(F8))
        hTds.append(np.ascontiguousarray(
            hp.reshape(R, NBT, 128, H).transpose(0, 2, 1, 3).reshape(
                R, 128, NBT * H)).astype(F16))            # node-major blocks
        lmls.append(lml)
        rcalls.append(rc)
        pmats.append(pmat)

    # weights in device layouts (shared across cores)
    # W1 fp8 DoubleRow stationary: [128p, 2i, (r*2+o)*128 + m]
    w1all8 = np.zeros((128, 2, R * 2 * 128), F8)
    b1all = np.zeros((128, R * 2), F32)
    for r in range(R):
        for o in range(2):
            for i in range(2):
                w1all8[:, i, (r * 2 + o) * 128:(r * 2 + o + 1) * 128] = \
                    W1[r, i * 128:(i + 1) * 128,
                       o * 128:(o + 1) * 128].astype(F8)
            b1all[:, r * 2 + o] = b1[r, o * 128:(o + 1) * 128]

    # w2 fp8 DoubleRow stationary, zero-column selection: chunk c of rank r
    # uses cols [(r*32+c)*32, ...+32) with w2 in column c, zero elsewhere.
    w2sel8 = np.zeros((128, 2, R * GSUB * GSUB), F8)
    for r in range(R):
        for i in range(2):
            w2c = w2[r, i * 128:(i + 1) * 128].astype(F8)
            for c in range(GSUB):
                w2sel8[:, i, (r * GSUB + c) * GSUB + c] = w2c

    # rank-proj: si 0..5 -> prA (sum, max, att), si 6..7 -> prB (mean)
    rows = [(0, 128), (128, 256), (512, 640), (640, 768),
            (768, 896), (896, 1024), (256, 384), (384, 512)]
    wpall = np.zeros((128, R * 8 * 256), F16)
    for r in range(R):
        for si, (a, b) in enumerate(rows):
            wpall[:, (r * 8 + si) * 256:(r * 8 + si + 1) * 256] = \
                Wp[r, a:b, :].astype(F16)
    bpbc = np.zeros((128, R * 256), F16)
    for r in range(R):
        bpbc[:, r * 256:(r + 1) * 256] = bp[r][None, :].astype(F16)

    lngbc = np.broadcast_to(ln_g.astype(F16), (128, R * 256)).copy()
    lnbbc = np.broadcast_to(ln_b.astype(F16), (128, R * 256)).copy()
    wf1 = np.zeros((128, 6 * 256), F16)
    for kb in range(6):
        wf1[:, kb * 256:(kb + 1) * 256] = Wf1[kb * 128:(kb + 1) * 128, :]
    bf1bc = np.broadcast_to(bf1.astype(F16), (128, 256)).copy()
    wf2 = np.zeros((128, 2), F16)
    for kb in range(2):
        wf2[:, kb] = Wf2[kb * 128:(kb + 1) * 128, 0]
    ident = np.eye(128, dtype=F32)
    ident16 = np.eye(128, dtype=F16)

    shared = dict(w1all8=w1all8, w2sel8=w2sel8, b1all=b1all, wpall=wpall,
                  bpbc=bpbc, lngbc=lngbc, lnbbc=lnbbc, wf1=wf1,
                  bf1bc=bf1bc, wf2=wf2, ident=ident, ident16=ident16,
                  maskc=maskc.reshape(128, NBT * 32))
    percore = [dict(hp8=hp8s[k], hTd=hTds[k], lml=lmls[k], rcall=rcalls[k],
                    pmat=pmats[k])
               for k in range(NCORES)]
    meta = dict(Ls=[int(x) for x in Ls],
                SUBNs=[int(x) for x in SUBNs], offs=[int(x) for x in offs],
                nbs=nbs, boffs=[int(x) for x in boffs], NBT=NBT,
                NLP=NLP, sigma=sigma, bf2=float(bf2[0]), perms=perms)
    return shared, percore, meta


# ---------------------------------------------------------------- device IR

def _build(ctx, tc, ins, out_ap, meta):
    import concourse.mybir as mybir

    nc = tc.nc
    dt = mybir.dt
    Act = mybir.ActivationFunctionType
    Alu = mybir.AluOpType
    AX = mybir.AxisListType
    DR = mybir.MatmulPerfMode.DoubleRow

    Ls, SUBNs, offs, nbs, boffs = (
        meta[k] for k in ("Ls", "SUBNs", "offs", "nbs", "boffs"))
    SUBN_MAX = max(SUBNs)
    L_MAX = max(Ls)
    NB_MAX = max(nbs)
    NBH_MAX = (NB_MAX + 1) // 2

    cpool = ctx.enter_context(tc.tile_pool(name="const", bufs=1))
    hpool = ctx.enter_context(tc.tile_pool(name="hp8", bufs=1))
    tpool = ctx.enter_context(tc.tile_pool(name="hT", bufs=3))
    mpool = ctx.enter_context(tc.tile_pool(name="mask", bufs=1))
    thpool = ctx.enter_context(tc.tile_pool(name="th8", bufs=1))
    spool = ctx.enter_context(tc.tile_pool(name="small", bufs=2))
    wapool = ctx.enter_context(tc.tile_pool(name="wall", bufs=2))
    rpool = ctx.enter_context(tc.tile_pool(name="rank", bufs=1))
    fpool = ctx.enter_context(tc.tile_pool(name="final", bufs=1))
    psx = ctx.enter_context(tc.tile_pool(name="psx", bufs=1, space="PSUM"))
    pss = ctx.enter_context(tc.tile_pool(name="pss", bufs=1, space="PSUM"))
    psp = ctx.enter_context(tc.tile_pool(name="psp", bufs=1, space="PSUM"))
    psr = ctx.enter_context(tc.tile_pool(name="psr", bufs=1, space="PSUM"))

    def const_tile(name):
        ap = ins[name]
        t = cpool.tile(list(ap.shape), ap.dtype, tag=name, name=name)
        nc.sync.dma_start(t[:], ap)
        return t

    w1all8 = const_tile("w1all8")    # [128, 2, R*2*128] fp8
    w2sel8 = const_tile("w2sel8")    # [128, 2, R*32*32] fp8
    b1all = const_tile("b1all")
    wpall = const_tile("wpall")
    bpbc = const_tile("bpbc")
    lngbc = const_tile("lngbc")
    lnbbc = const_tile("lnbbc")
    wf1 = const_tile("wf1")
    bf1bc = const_tile("bf1bc")
    wf2 = const_tile("wf2")
    ident = const_tile("ident")
    ident16 = const_tile("ident16")
    lml = const_tile("lml")          # [32, R*NSUB] f32
    rcall = const_tile("rcall")
    pmat = const_tile("pmat")        # [128, R*2*2*128] fp16
    maskd = ins["maskc"]             # [128, NBT*32] fp16 dram (streamed)

    hp8d = ins["hp8"]   # [R, 128, 2, NLP] fp8 dram
    hTd = ins["hTd"]    # [R, 128, NBT*256] fp16 dram
    NBP_MAX = (NB_MAX + 15) // 16 * 16
    scr = nc.dram_tensor(f"scratch{nc.next_id()}", (2, NBP_MAX * 128),
                         dt.float16, kind="Internal").ap()

    state = [fpool.tile([128, 3 * 256], dt.float16, tag=f"state{gh}",
                        name=f"state{gh}")
             for gh in range(2)]

    T = R * NSUB                     # tiles, s-major: t = s*R + r
    hp8_t, hT_t, th8_t, psS_t, wall_t, mask_t = {}, {}, {}, {}, {}, {}
    pools_r = {}

    def new_rank_pools(r):
        SM = [rpool.tile([128, 256], dt.float16, tag=f"sm{r}_{i}",
                         name=f"sm{r}_{i}") for i in range(2)]
        MX = [rpool.tile([128, 256], dt.float16, tag=f"mx{r}_{i}",
                         name=f"mx{r}_{i}") for i in range(2)]
        AT = [rpool.tile([128, 256], dt.float16, tag=f"at{r}_{i}",
                         name=f"at{r}_{i}") for i in range(2)]
        return SM, MX, AT

    def dma_hp8(t):
        s, r = t // R, t % R
        SUBN, off = SUBNs[s], offs[s]
        hp8 = hpool.tile([128, 2, SUBN_MAX], dt.float8e4, tag=f"hp8{t % 2}",
                         name=f"hp8{t % 2}", bufs=1)
        nc.sync.dma_start(hp8[:, :, :SUBN], hp8d[r, :, :, off:off + SUBN])
        hp8_t[t] = hp8

    def dma_mask(s):
        nb, boff = nbs[s], boffs[s]
        mk = mpool.tile([128, NB_MAX * 32], dt.float16, tag=f"mask{s % 2}",
                        name=f"mask{s % 2}", bufs=1)
        nc.sync.dma_start(mk[:, :nb * 32], maskd[:, boff * 32:(boff + nb) * 32])
        mask_t[s] = mk

    def dma_hT(t):
        s, r = t // R, t % R
        nb, boff = nbs[s], boffs[s]
        hT = tpool.tile([128, NB_MAX * 256], dt.float16, tag="hT", name="hT")
        nc.sync.dma_start(hT[:, :nb * 256],
                          hTd[r, :, boff * 256:(boff + nb) * 256])
        hT_t[t] = hT

    def pe_tile(t):
        """W1 matmuls (fp8 DoubleRow) + tanh into fp8 th tile."""
        s, r = t // R, t % R
        SUBN = SUBNs[s]
        hp8 = hp8_t[t]
        th8 = thpool.tile([128, 2, SUBN_MAX], dt.float8e4, tag=f"th8{t % 2}",
                          name=f"th8{t % 2}", bufs=1)
        for o in range(2):
            base = (r * 2 + o) * 128
            c0 = 0
            while c0 < SUBN:
                c1 = min(SUBN, c0 + 1024)
                px = psx.tile([128, 1024], dt.float32,
                              tag=f"psx{(c0 // 1024) % 2}", bufs=1)
                m0 = min(c0 + 512, c1)
                nc.tensor.matmul(px[:, :m0 - c0],
                                 w1all8[:, :, base:base + 128],
                                 hp8[:, :, c0:m0], start=True, stop=True,
                                 perf_mode=DR)
                if c1 > m0:
                    nc.tensor.matmul(px[:, 512:512 + c1 - m0],
                                     w1all8[:, :, base:base + 128],
                                     hp8[:, :, m0:c1], start=True, stop=True,
                                     perf_mode=DR)
                nc.scalar.activation(th8[:, o, c0:c1], px[:, :c1 - c0],
                                     Act.Tanh,
                                     bias=b1all[:, r * 2 + o:r * 2 + o + 1])
                c0 = c1
        th8_t[t] = th8

    def w2_tile(t):
        """Score matmuls for tile t (issued one stage later)."""
        s, r = t // R, t % R
        L = Ls[s]
        th8 = th8_t.pop(t)
        pssT = pss.tile([32, 2, 256], dt.float32, tag="pss", bufs=1)
        psS = pssT[:, t % 2, :L_MAX]
        for c in range(GSUB):
            sel = (r * GSUB + c) * GSUB
            nc.tensor.matmul(psS[:, :L], w2sel8[:, :, sel:sel + GSUB],
                             th8[:, :, c * L:(c + 1) * L],
                             start=(c == 0), stop=(c == GSUB - 1),
                             perf_mode=DR, skip_group_check=True)
        psS_t[t] = psS

    def softmax_part(t):
        """Softmax for tile t, then eT + w_all construction."""
        s, r = t // R, t % R
        L, SUBN, nb, boff = Ls[s], SUBNs[s], nbs[s], boffs[s]
        psS = psS_t.pop(t)
        negm = spool.tile([32, 1], dt.float32, tag="negm")
        nc.vector.tensor_reduce(negm[:], psS[:, :L], axis=AX.X, op=Alu.max,
                                negate=True)
        e16 = spool.tile([32, L_MAX], dt.float16, tag="e16", bufs=1)
        nc.scalar.activation(e16[:, :L], psS[:, :L], Act.Exp, bias=negm[:])
        den = spool.tile([32, 1], dt.float32, tag="den")
        nc.vector.tensor_reduce(den[:], e16[:, :L], axis=AX.X, op=Alu.add)
        idx = r * NSUB + s
        fix = spool.tile([32, 1], dt.float32, tag="fix")
        nc.scalar.activation(fix[:], negm[:], Act.Exp,
                             bias=lml[:, idx:idx + 1])
        dent = spool.tile([32, 1], dt.float32, tag="dent")
        nc.vector.tensor_tensor(dent[:], den[:], fix[:], op=Alu.subtract)
        rden = spool.tile([32, 1], dt.float32, tag="rden")
        nc.vector.reciprocal(rden[:], dent[:])
        wsb = spool.tile([32, L_MAX], dt.float16, tag="wsb", bufs=1)
        nc.vector.tensor_scalar_mul(wsb[:, :L], e16[:, :L], rden[:])
        # store graph-major rows to DRAM (node order), then fold back to
        # [128 x nb] columns (node b*128+p) via the xbar transpose
        nc.sync.dma_start(
            scr[t % 2:t % 2 + 1, :SUBN].rearrange("x (c f) -> (x c) f", f=L),
            wsb[:, :L])
        nbp = (nb + 15) // 16 * 16
        eT = spool.tile([128, NBP_MAX], dt.float16, tag="eT")
        nc.sync.dma_start_transpose(
            eT[:, :nbp],
            scr[t % 2:t % 2 + 1, :nbp * 128].rearrange("x (b q) -> (x b) q",
                                                       q=128))
        # w_all[:, b*64:(b+1)*64]: [mask | mask*eT[:,b]]
        w_all = wapool.tile([128, NB_MAX * 64], dt.float16, tag="wall")
        wv = w_all[:, :nb * 64].rearrange("p (b j) -> p b j", j=64)
        mv = mask_t[s][:, :nb * 32].rearrange("p (b j) -> p b j", j=32)
        nc.vector.tensor_copy(wv[:, :, 0:32], mv)
        ev = eT[:, :nb].unsqueeze(-1).to_broadcast([128, nb, 32])
        nc.vector.tensor_tensor(wv[:, :, 32:64], mv, ev, op=Alu.mult)
        wall_t[t] = w_all

    def max_tile(t):
        s, r = t // R, t % R
        L, SUBN = Ls[s], SUBNs[s]
        g0 = s * GSUB
        if r not in pools_r:
            pools_r[r] = new_rank_pools(r)
        SM, MX, AT = pools_r[r]
        hp8 = hp8_t.pop(t)
        for i in range(2):
            hv = hp8[:, i, :SUBN].rearrange("p (g l) -> p g l", l=L)
            nc.vector.tensor_reduce(MX[i][:, g0:g0 + GSUB], hv,
                                    axis=AX.X, op=Alu.max)

    def pool_mm(t):
        """Sum+att pooling matmuls for tile t, plus transposes back."""
        s, r = t // R, t % R
        nb = nbs[s]
        g0 = s * GSUB
        w_all = wall_t.pop(t)
        hT = hT_t.pop(t)
        SM, MX, AT = pools_r[r]
        pp = psp.tile([64, 256], dt.float32, tag="pp")
        for b in range(nb):
            nc.tensor.matmul(pp[:], w_all[:, b * 64:(b + 1) * 64],
                             hT[:, b * 256:(b + 1) * 256],
                             start=(b == 0), stop=(b == nb - 1))
        pc = spool.tile([64, 256], dt.float32, tag="pc")
        nc.vector.tensor_copy(pc[:], pp[:])
        ptr = psp.tile([128, 128], dt.float32, tag="ptr", bufs=1)
        for hh in range(2):
            nc.tensor.matmul(ptr[:, hh * 64:(hh + 1) * 64],
                             pc[:, hh * 128:(hh + 1) * 128],
                             ident[:64, :64], is_transpose=True)
            nc.scalar.copy(SM[hh][:, g0:g0 + GSUB],
                           ptr[:, hh * 64:hh * 64 + 32])
            nc.scalar.copy(AT[hh][:, g0:g0 + GSUB],
                           ptr[:, hh * 64 + 32:hh * 64 + 64])

    def rank_tail(r):
        SM, MX, AT = pools_r.pop(r)
        pools6 = [SM[0], SM[1], MX[0], MX[1], AT[0], AT[1]]
        t16 = []
        for gh in range(2):
            prT = psr.tile([128, 2, 256], dt.float32, tag="prAB", bufs=1)
            prA, prB = prT[:, 0, :], prT[:, 1, :]
            for si in range(6):
                nc.tensor.matmul(prA[:], pools6[si][:, gh * 128:(gh + 1) * 128],
                                 wpall[:, (r * 8 + si) * 256:(r * 8 + si + 1) * 256],
                                 start=(si == 0), stop=(si == 5))
            for si in (6, 7):
                nc.tensor.matmul(prB[:],
                                 pools6[si - 6][:, gh * 128:(gh + 1) * 128],
                                 wpall[:, (r * 8 + si) * 256:(r * 8 + si + 1) * 256],
                                 start=(si == 6), stop=(si == 7))
            tmp = fpool.tile([128, 256], dt.float32, tag="prtmp", bufs=1)
            nc.vector.tensor_scalar_mul(tmp[:], prB[:],
                                        rcall[:, r * 2 + gh:r * 2 + gh + 1])
            nc.vector.tensor_tensor(tmp[:], tmp[:], prA[:], op=Alu.add)
            tg = fpool.tile([128, 256], dt.float16, tag=f"t16_{gh}", bufs=1,
                            name=f"t16_{gh}")
            nc.vector.tensor_copy(tg[:], tmp[:])
            t16.append(tg)
        # un-permute graph partitions: state[ghp] = sum_gh pmat[gh->ghp]^T @ t16[gh]
        for ghp in range(2):
            prT = psr.tile([128, 2, 256], dt.float32, tag="prAB", bufs=1)
            pperm = prT[:, 0, :]
            for gh in range(2):
                col = ((r * 2 + gh) * 2 + ghp) * 128
                nc.tensor.matmul(pperm[:], pmat[:, col:col + 128], t16[gh][:],
                                 start=(gh == 0), stop=(gh == 1))
            nc.vector.tensor_tensor(state[ghp][:, r * 256:(r + 1) * 256],
                                    pperm[:], bpbc[:, r * 256:(r + 1) * 256],
                                    op=Alu.add)

    # ------------------------------------------------- software pipeline
    # stage t: dma_hp8(t+1) | w2(t-1) -> softmax(t-1) | pe(t) | max(t) |
    #          pool_mm(t-1) | dma_hT(t+1)
    # w2->softmax run early so the w_all DMA-fold chain for tile t-1
    # completes while pe(t) streams; pool_mm(t-1) then runs stall-free.
    # dma_hT(t+1) is issued after pool_mm(t-1) so the 2-buffer hT rotation
    # never reclaims a buffer before its reader is on the queue.
    dma_mask(0)
    dma_hp8(0)
    dma_hT(0)
    for t in range(T):
        if t + 1 < T:
            dma_hp8(t + 1)
            if (t + 1) % R == 0:
                dma_mask((t + 1) // R)
        if t > 0:
            w2_tile(t - 1)
            softmax_part(t - 1)
        pe_tile(t)
        max_tile(t)
        if t > 1:
            pool_mm(t - 2)
        if t + 1 < T:
            dma_hT(t + 1)
    w2_tile(T - 1)
    softmax_part(T - 1)
    if T > 1:
        pool_mm(T - 2)
    pool_mm(T - 1)
    for r in range(R):
        rank_tail(r)

    # ------------------------------------- final MLP (LN->SiLU->L->SiLU->L)
    D = 3 * 256
    for gh in range(2):
        st = state[gh]
        mu = fpool.tile([128, 1], dt.float32, tag="mu")
        nc.vector.tensor_reduce(mu[:], st[:], axis=AX.X, op=Alu.add)
        nc.vector.tensor_scalar_mul(mu[:], mu[:], 1.0 / D)
        xm = fpool.tile([128, D], dt.float32, tag="xm")
        nc.vector.tensor_scalar(xm[:], st[:], mu[:], None, op0=Alu.subtract)
        y = fpool.tile([128, D], dt.float32, tag="y")
        varsum = fpool.tile([128, 1], dt.float32, tag="vs")
        nc.scalar.activation(y[:], xm[:], Act.Square, accum_out=varsum[:])
        sdv = fpool.tile([128, 1], dt.float32, tag="sdv")
        nc.vector.tensor_scalar(sdv[:], varsum[:], 1.0 / D, 1e-5,
                                op0=Alu.mult, op1=Alu.add)
        nc.scalar.activation(sdv[:], sdv[:], Act.Sqrt)
        rstd = fpool.tile([128, 1], dt.float32, tag="rstd")
        nc.vector.reciprocal(rstd[:], sdv[:])
        nc.vector.tensor_scalar_mul(y[:], xm[:], rstd[:])
        nc.vector.tensor_tensor(y[:], y[:], lngbc[:], op=Alu.mult)
        nc.vector.tensor_tensor(y[:], y[:], lnbbc[:], op=Alu.add)
        nc.scalar.activation(xm[:], y[:], Act.Sigmoid)
        nc.vector.tensor_mul(xm[:], xm[:], y[:])
        x2 = xm

        pf = psx.tile([128, 1024], dt.float32, tag="psx0", bufs=1)
        for kb in range(6):
            pt = psx.tile([128, 1024], dt.float32, tag="psx1", bufs=1)
            nc.tensor.matmul(pt[:, :128], x2[:, kb * 128:(kb + 1) * 128],
                             ident[:], is_transpose=True)
            xT = fpool.tile([128, 128], dt.float16, tag=f"xT{kb}")
            nc.scalar.copy(xT[:], pt[:, :128])
            nc.tensor.matmul(pf[:, :256], xT[:], wf1[:, kb * 256:(kb + 1) * 256],
                             start=(kb == 0), stop=(kb == 5))
        xf = fpool.tile([128, 256], dt.float32, tag="xf")
        nc.vector.tensor_tensor(xf[:], pf[:, :256], bf1bc[:], op=Alu.add)
        xs = fpool.tile([128, 256], dt.float32, tag="xs")
        nc.scalar.activation(xs[:], xf[:], Act.Sigmoid)
        nc.vector.tensor_mul(xf[:], xf[:], xs[:])

        poT = psr.tile([128, 2, 256], dt.float32, tag="prAB", bufs=1)
        po = poT[:, 0, :]
        for kb in range(2):
            pt = psx.tile([128, 1024], dt.float32, tag="psx1", bufs=1)
            nc.tensor.matmul(pt[:, :128], xf[:, kb * 128:(kb + 1) * 128],
                             ident[:], is_transpose=True)
            xT = fpool.tile([128, 128], dt.float16, tag=f"xfT{kb}")
            nc.scalar.copy(xT[:], pt[:, :128])
            nc.tensor.matmul(po[:, :1], xT[:], wf2[:, kb:kb + 1],
                             start=(kb == 0), stop=(kb == 1))
        osb = fpool.tile([128, 1], dt.float32, tag=f"osb{gh}")
        nc.vector.tensor_scalar_add(osb[:], po[:, :1], meta["bf2"])
        nc.sync.dma_start(out_ap[gh], osb[:])


# ---------------------------------------------------------------- driver

def _make_nc(shared, percore, meta, reps=1):
    import concourse.bacc as bacc
    import concourse.mybir as mybir
    from concourse import tile

    nc = bacc.Bacc("TRN2", target_bir_lowering=False, debug=False,
                   enable_asserts=False, num_devices=NCORES)
    ins = {}
    for name, arr in {**shared, **percore[0]}.items():
        ins[name] = nc.dram_tensor(name, arr.shape,
                                   mybir.dt.from_np(arr.dtype),
                                   kind="ExternalInput").ap()
    out_ap = nc.dram_tensor("out", (2, 128, 1), mybir.dt.float32,
                            kind="ExternalOutput").ap()
    with tile.TileContext(nc, trace_sim=False) as t:
        for _ in range(reps):
            with ExitStack() as ctx:
                _build(ctx, t, ins, out_ap, meta)
    nc.compile()
    return nc


LAST_EXEC_NS = None


def _gather_out(results, meta):
    out = np.zeros((G,), F32)
    for k in range(NCORES):
        out[k * GLOC:(k + 1) * GLOC] = results[k]["out"].reshape(256)
    return out


def _run_timed(nc, in_maps, reps):
    """Run via pjrt with inputs device-resident; derive per-rep device time
    from the marginal async per-call time (tunnel overhead ~1ms cancels)."""
    import time
    import jax
    from jax.sharding import Mesh, PartitionSpec, NamedSharding
    from jax.experimental.shard_map import shard_map
    from concourse import bass2jax
    import concourse.mybir as mybir

    bass2jax.install_neuronx_cc_hook()
    n_cores = len(in_maps)
    in_names, out_names, out_avals = [], [], []
    for alloc in nc.m.functions[0].allocations:
        if not isinstance(alloc, mybir.MemoryLocationSet):
            continue
        if not alloc.memorylocations:
            continue
        name = alloc.memorylocations[0].name
        pname = (nc.partition_id_tensor.name
                 if nc.partition_id_tensor else None)
        if alloc.kind == "ExternalInput":
            if name != pname:
                in_names.append(name)
        elif alloc.kind == "ExternalOutput":
            out_names.append(name)
            out_avals.append(jax.core.ShapedArray(
                tuple(alloc.tensor_shape), mybir.dt.np(alloc.dtype)))
    n_params = len(in_names)
    in_names = in_names + out_names
    if nc.partition_id_tensor is not None:
        in_names.append(nc.partition_id_tensor.name)

    def _body(*args):
        operands = list(args)
        if nc.partition_id_tensor is not None:
            operands.append(bass2jax.partition_id_tensor())
        outs = bass2jax._bass_exec_p.bind(
            *operands, out_avals=tuple(out_avals), in_names=tuple(in_names),
            out_names=tuple(out_names), lowering_input_output_aliases=(),
            sim_require_finite=True, sim_require_nnan=True, nc=nc)
        return tuple(outs)

    devices = jax.devices()[:n_cores]
    mesh = Mesh(np.asarray(devices), ("core",))
    nio = n_params + len(out_names)
    sharded = jax.jit(shard_map(_body, mesh=mesh,
                                in_specs=(PartitionSpec("core"),) * nio,
                                out_specs=(PartitionSpec("core"),) * len(out_names),
                                check_rep=False), keep_unused=True)
    sh = NamedSharding(mesh, PartitionSpec("core"))
    concat_in = [jax.device_put(np.concatenate(
        [np.asarray(in_maps[c][nm]) for c in range(n_cores)], axis=0), sh)
        for nm in in_names[:n_params]]
    zeros = [jax.device_put(np.zeros((n_cores * a.shape[0],) + a.shape[1:],
                                     a.dtype), sh) for a in out_avals]
    outs = sharded(*concat_in, *zeros)
    jax.block_until_ready(outs)

    best = None
    if reps > 1:
        def async_total(n):
            jax.block_until_ready(sharded(*concat_in, *zeros))
            t0 = time.perf_counter()
            rs = [sharded(*concat_in, *zeros) for _ in range(n)]
            jax.block_until_ready(rs)
            return time.perf_counter() - t0
        for _ in range(3):
            marg = (async_total(24) - async_total(4)) / 20.0
            if best is None or marg < best:
                best = marg
        best = max(0.0, (best - 1.03e-3)) / reps  # subtract dispatch overhead
    out_np = [np.asarray(o) for o in outs]
    results = []
    for c in range(n_cores):
        m = {}
        for i, nm in enumerate(out_names):
            per = out_avals[i].shape[0]
            m[nm] = out_np[i][c * per:(c + 1) * per]
        results.append(m)
    return results, best


def kernel(**inputs):
    global LAST_EXEC_NS
    import os
    shared, percore, meta = _prep(inputs)
    in_maps = [{**shared, **percore[k]} for k in range(NCORES)]

    reps = int(os.environ.get("KERNEL_TIME_REPS", "0"))
    if reps > 1:
        # timing mode: unroll the body so device time dominates the tunnel
        try:
            nc = _make_nc(shared, percore, meta, reps=reps)
            results, dev_s = _run_timed(nc, in_maps, reps)
            if dev_s is not None:
                LAST_EXEC_NS = int(dev_s * 1e9)
                print(f"measured device time: {dev_s*1e6:.1f} us/rep")
            return _gather_out(results, meta)
        except Exception as e:
            print(f"timing mode failed ({e!r}); falling back to single run")

    nc = _make_nc(shared, percore, meta)
    from concourse import bass_utils
    res = bass_utils.run_bass_kernel_spmd(nc, in_maps,
                                          core_ids=list(range(NCORES)))
    if getattr(res, "exec_time_ns", None):
        LAST_EXEC_NS = int(res.exec_time_ns)
    return _gather_out(res.results, meta)
